# revision 11
# baseline (speedup 1.0000x reference)
"""TRN2 Bass kernel for nn_BalancedHamiltonLayer.

Math: out[n,k,j] = sum_{r,s,i} x[n,s,i] * factors_B[r,j,i] * H(A)[r,k,s] + bias
collapses to a single dense matmul  out = x2d @ W + bias  with
W[(s,i),(k,j)] = sum_r H[r,k,s] * B[r,j,i]  (a 1024x1024 matrix folded on host
in float64).

Sharding: data-parallel over the 8192 token rows across 8 NeuronCores
(1024 rows each); W replicated.  The matmul runs in fp16 on the PE
(full-rate, FWL weight loads, ~5e-4 relative error; fp32 PSUM
accumulation).  x is passed pre-transposed per core as
[m2, f_in, 256 tokens] so lhsT tiles load contiguously (512B bursts).
bias is added on the host during the gather.
"""

import numpy as np
import concourse.bacc as bacc
import concourse.mybir as mybir
import concourse.tile as tile
from concourse.bass_utils import run_bass_kernel_spmd

B, T, D = 4, 2048, 1024
RANK, FACTOR, SUB = 8, 64, 4
S = 4 * SUB  # 16
NCORES = 8
NTOK = B * T // NCORES  # 1024 token rows per core
P = 128
KT = D // P     # 8 contraction chunks
MT = NTOK // P  # 8 token tiles per core
M2 = MT // 2    # x DMA granularity: 256-token slabs
NH = 512        # f_out half (one PSUM bank)

_cached_nc = None


def build_module():
    global _cached_nc
    if _cached_nc is not None:
        return _cached_nc
    nc = bacc.Bacc("TRN2", target_bir_lowering=False, debug=False)
    xT = nc.dram_tensor("xT", [M2, D, 2 * P], mybir.dt.float16, kind="ExternalInput").ap()
    w = nc.dram_tensor("w", [D, D], mybir.dt.float16, kind="ExternalInput").ap()
    out = nc.dram_tensor("out", [NTOK, D], mybir.dt.float32, kind="ExternalOutput").ap()

    with tile.TileContext(nc) as tc:
        with (
            tc.tile_pool(name="wp", bufs=1) as wp,
            tc.tile_pool(name="xp", bufs=1) as xp,
            tc.tile_pool(name="op", bufs=4) as op,
            tc.tile_pool(name="ps", bufs=3, space="PSUM") as ps,
        ):
            # PE HAM pre-warm: matmuls on a zeroed SBUF tile accumulate +0
            # into the first real PSUM group while data DMAs are in flight,
            # so the clock gate is at 2.4 GHz when real matmuls start.
            g = xp.tile([P, NH], mybir.dt.float16, tag="warm", name="g")
            nc.gpsimd.memset(g[:], 0.0)

            # DMA issue order = consumption order: first blocks of x and W
            # in small chunks (early semaphore fire), then the rest.
            xt = {}
            xt[0] = xp.tile([P, KT, 2 * P], mybir.dt.float16, tag="x0", name="xt0")
            nc.sync.dma_start(
                xt[0][:, 0, :], xT[0, 0:P, :]
            )
            wt = {}
            wt[0] = wp.tile([P, 2 * NH], mybir.dt.float16, tag="w0", name="wt0")
            nc.sync.dma_start(wt[0][:, :NH], w[0:P, :NH])
            nc.sync.dma_start(
                xt[0][:, 1:, :],
                xT[0, P:, :].rearrange("(k p) t -> p k t", p=P),
            )
            nc.sync.dma_start(wt[0][:, NH:], w[0:P, NH:])
            xt[1] = xp.tile([P, KT, 2 * P], mybir.dt.float16, tag="x1", name="xt1")
            nc.sync.dma_start(xt[1][:], xT[1].rearrange("(k p) t -> p k t", p=P))
            for k in range(1, KT):
                t = wp.tile([P, 2 * NH], mybir.dt.float16, tag=f"w{k}", name=f"wt{k}")
                nc.sync.dma_start(t[:], w[k * P:(k + 1) * P, :])
                wt[k] = t
            for m2 in range(2, M2):
                t = xp.tile([P, KT, 2 * P], mybir.dt.float16, tag=f"x{m2}", name=f"xt{m2}")
                nc.sync.dma_start(t[:], xT[m2].rearrange("(k p) t -> p k t", p=P))
                xt[m2] = t

            def emit_out(m, pt):
                for n in range(2):
                    o = op.tile([P, NH], mybir.dt.float32, tag="o", name="o")
                    nc.vector.tensor_copy(o[:], pt[n][:])
                    nc.sync.dma_start(
                        out[m * P:(m + 1) * P, n * NH:(n + 1) * NH], o[:]
                    )

            def xs_of(m):
                return xt[m // 2][:, :, (m % 2) * P:(m % 2 + 1) * P]

            with nc.named_scope("mm"):
                # Phase 1: m=0,1,2 k-interleaved — compute rate matches the
                # W-chunk arrival rate, so the PE doesn't stall on w[k] sems.
                NP1 = 3
                pts = {
                    m: {
                        n: ps.tile([P, NH], mybir.dt.float32, tag=f"ps{n}", name=f"pt{m}_{n}")
                        for n in range(2)
                    }
                    for m in range(NP1)
                }
                NWARM = 6
                for i in range(NWARM):
                    nc.tensor.matmul(
                        pts[0][0][:], g[:, :P], g[:], start=(i == 0), stop=False
                    )
                for k in range(KT):
                    for m in range(NP1):
                        for n in range(2):
                            nc.tensor.matmul(
                                pts[m][n][:],
                                xs_of(m)[:, k, :],
                                wt[k][:, n * NH:(n + 1) * NH],
                                start=(k == 0 and not (m == 0 and n == 0)),
                                stop=(k == KT - 1),
                            )
                for m in range(NP1):
                    emit_out(m, pts[m])

                # Phase 2: k-contiguous per m-tile (PE stays warm, dense)
                for m in range(NP1, MT):
                    pt = {
                        n: ps.tile([P, NH], mybir.dt.float32, tag=f"ps{n}", name=f"pt{n}")
                        for n in range(2)
                    }
                    for k in range(KT):
                        for n in range(2):
                            nc.tensor.matmul(
                                pt[n][:],
                                xs_of(m)[:, k, :],
                                wt[k][:, n * NH:(n + 1) * NH],
                                start=(k == 0),
                                stop=(k == KT - 1),
                            )
                    emit_out(m, pt)
    nc.compile()
    _cached_nc = nc
    return nc


def _construct_hamilton(A):
    # A: [rank, 4, sub, sub] -> [rank, 4*sub, 4*sub]
    r, i, j, k = A[:, 0], A[:, 1], A[:, 2], A[:, 3]
    return np.concatenate(
        [
            np.concatenate([r, -i, -j, -k], axis=2),
            np.concatenate([i, r, -k, j], axis=2),
            np.concatenate([j, k, r, -i], axis=2),
            np.concatenate([k, -j, i, r], axis=2),
        ],
        axis=1,
    )


def build_in_maps(x, A, factors_B):
    H = _construct_hamilton(np.asarray(A, dtype=np.float64))  # [r, k, s]
    Bf = np.asarray(factors_B, dtype=np.float64)  # [r, j, i]
    # W[(s,i),(k,j)] = sum_r H[r,k,s] * B[r,j,i]
    W = np.einsum("rks,rji->sikj", H, Bf).reshape(D, D).astype(np.float16)

    x2 = np.asarray(x, dtype=np.float16).reshape(NCORES, NTOK, D)
    in_maps = []
    for c in range(NCORES):
        # [NTOK, D] -> [M2, 256, D] -> [M2, D, 256]
        xs = np.ascontiguousarray(x2[c].reshape(M2, 2 * P, D).transpose(0, 2, 1))
        in_maps.append({"xT": xs, "w": W})
    return in_maps


def kernel(x, A, factors_B, bias):
    nc = build_module()
    in_maps = build_in_maps(x, A, factors_B)
    br = run_bass_kernel_spmd(nc, in_maps, core_ids=list(range(NCORES)))
    out = np.concatenate([r["out"] for r in br.results], axis=0)
    out = out + np.asarray(bias, dtype=np.float32)[None, :]
    return out.reshape(B, T, D).astype(np.float32)
